# revision 10
# baseline (speedup 1.0000x reference)
"""Trainium2 Bass kernel for single-token causal self-attention with an
int8-quantized KV cache (dequant -> attend -> requant), sharded over 8
NeuronCores.

Sharding: tensor-parallel over heads. Core c owns heads {2c, 2c+1} and all 16
batch rows. W_attn is column-sliced, W_proj row-sliced; each core emits a
partial y which the host sums (the "all-reduce" realized at unshard time).
The KV cache, scales, and quantized outputs are head-sliced so each core
streams 1/8 of the ~1.4GB of HBM traffic.
"""

import math
from contextlib import ExitStack

import numpy as np

import concourse.bass as bass
import concourse.tile as tile
import concourse.mybir as mybir
from concourse import bacc
from concourse.bass_utils import run_bass_kernel_spmd
from concourse.masks import make_identity

P = 128
B = 16
NH = 16
HS = 128
C = 2048
HPC = 2  # heads per core
NCORES = 8
F32 = mybir.dt.float32
I32 = mybir.dt.int32
I8 = mybir.dt.int8

_BUILD_CACHE = {}


def build(nt=32):
    """Build the per-core Bass program. nt = number of 128-token tiles
    (T_total = nt*128; full problem is nt=32 -> 4096)."""
    T = nt * P
    NPAIR = HPC * B  # 32 (head, batch) pairs per core
    M3 = 3 * HPC  # qkv m-tiles of 128: [q_h0 q_h1 k_h0 k_h1 v_h0 v_h1]
    CK = C // P  # 16 contraction tiles for the qkv projection
    ISQ = 1.0 / math.sqrt(HS)

    nc = bacc.Bacc("TRN2", target_bir_lowering=False, debug=False,
                   num_devices=NCORES)

    x = nc.dram_tensor("x", [B, C], F32, kind="ExternalInput").ap()
    wqkv = nc.dram_tensor("wqkv", [C, M3 * P], F32, kind="ExternalInput").ap()
    bqkv = nc.dram_tensor("bqkv", [M3 * P], F32, kind="ExternalInput").ap()
    wproj = nc.dram_tensor("wproj", [HPC * P, C], F32, kind="ExternalInput").ap()
    # caches pre-padded to T rows on host (row T-1 is zero, overwritten with
    # the new token's k/v on device); pair index = h_local*B + b
    kq = nc.dram_tensor("kq", [NPAIR, T, HS], I32, kind="ExternalInput").ap()
    vq = nc.dram_tensor("vq", [NPAIR, T, HS], I32, kind="ExternalInput").ap()
    # packed scales, [pair, {k,v}, T]; slot T-1 pre-set to 1.0 on host
    sc = nc.dram_tensor("sc", [NPAIR, 2, T], F32, kind="ExternalInput").ap()

    y_part = nc.dram_tensor("y_part", [B, C], F32, kind="ExternalOutput").ap()
    kqo_d = nc.dram_tensor("kqo", [NPAIR, T, HS], I8, kind="ExternalOutput").ap()
    kso_d = nc.dram_tensor("kso", [NPAIR, T], F32, kind="ExternalOutput").ap()
    vqo_d = nc.dram_tensor("vqo", [NPAIR, T, HS], I8, kind="ExternalOutput").ap()
    vso_d = nc.dram_tensor("vso", [NPAIR, T], F32, kind="ExternalOutput").ap()

    with tile.TileContext(nc) as tc, ExitStack() as ctx:
        const = ctx.enter_context(tc.tile_pool(name="const", bufs=1))

        ident = const.tile([P, P], F32)
        make_identity(nc, ident)
        ones_col = const.tile([P, 1], F32)
        nc.gpsimd.memset(ones_col, 1.0)
        ones_row = const.tile([1, P], F32)
        nc.gpsimd.memset(ones_row, 1.0)

        # all scales resident: [P, pair, kv, j]
        sc_sb = const.tile([P, NPAIR, 2, nt], F32)
        nc.sync.dma_start(sc_sb, sc.rearrange("r v (j p) -> p r v j", p=P))
        # W_proj resident: [P, head, C]
        wp_sb = const.tile([P, HPC, C], F32)
        nc.sync.dma_start(wp_sb, wproj.rearrange("(h p) n -> p h n", p=P))

        qkvT = const.tile([P, M3, B], F32)  # [hs, mtile, batch]
        knew = const.tile([B, HPC, HS], F32)  # new-token rows, batch on part
        vnew = const.tile([B, HPC, HS], F32)
        yh = const.tile([P, HPC, B], F32)  # per-head attn outputs

        # ---- phase 0: qkv projection ----
        with tc.tile_pool(name="p0", bufs=1) as p0, \
             tc.tile_pool(name="p0ps", bufs=2, space="PSUM") as p0ps:
            xt = p0.tile([P, CK, B], F32)
            for k in range(CK):
                nc.sync.dma_start(xt[:, k, :],
                                  x[:, k * P:(k + 1) * P].rearrange("b p -> p b"))
            wsb = p0.tile([P, CK, M3 * P], F32)
            nc.sync.dma_start(wsb, wqkv.rearrange("(k p) m -> p k m", p=P))
            bq = p0.tile([P, M3], F32)
            nc.sync.dma_start(bq, bqkv.rearrange("(m p) -> p m", p=P))

            for m in range(M3):
                ps = p0ps.tile([P, B], F32, tag="proj")
                for k in range(CK):
                    nc.tensor.matmul(ps, lhsT=wsb[:, k, m * P:(m + 1) * P],
                                     rhs=xt[:, k, :],
                                     start=(k == 0), stop=(k == CK - 1))
                nc.scalar.activation(qkvT[:, m, :], ps,
                                     mybir.ActivationFunctionType.Identity,
                                     bias=bq[:, m:m + 1], scale=1.0)
            # new-token k/v as [batch, hs] rows via PE transpose
            for i, m in enumerate(range(HPC, M3)):
                pt = p0ps.tile([B, P], F32, tag="tr")
                nc.tensor.transpose(pt, qkvT[:, m, :], ident)
                dst = knew if i < HPC else vnew
                nc.scalar.copy(dst[:, i % HPC, :], pt)

        # ---- per-pair pipeline (3-stage software pipeline) ----
        pair_ctx = ctx.enter_context(ExitStack())
        kp = pair_ctx.enter_context(tc.tile_pool(name="kq", bufs=2))
        vp = pair_ctx.enter_context(tc.tile_pool(name="vq", bufs=2))
        ktsp = pair_ctx.enter_context(tc.tile_pool(name="kts", bufs=3))
        qop = pair_ctx.enter_context(tc.tile_pool(name="qout", bufs=2))
        sm = pair_ctx.enter_context(tc.tile_pool(name="small", bufs=3))
        ktps = pair_ctx.enter_context(tc.tile_pool(name="ktps", bufs=2, space="PSUM"))
        attps = pair_ctx.enter_context(tc.tile_pool(name="attps", bufs=2, space="PSUM"))
        yzps = pair_ctx.enter_context(tc.tile_pool(name="yzps", bufs=3, space="PSUM"))

        AL = mybir.AluOpType
        st = {}  # pipeline state per pair idx

        def front(i):
            h, b = divmod(i, B)
            s = {}
            ks_ap = sc_sb[:, i, 0, :]
            vs_ap = sc_sb[:, i, 1, :]
            kq_sb = kp.tile([P, nt, HS], F32, tag="kq")
            nc.gpsimd.dma_start(kq_sb, kq[i].rearrange("(j p) d -> p j d", p=P))
            nc.sync.dma_start(kq_sb[P - 1:P, nt - 1, :], knew[b:b + 1, h, :])
            vq_sb = vp.tile([P, nt, HS], F32, tag="vq")
            nc.gpsimd.dma_start(vq_sb, vq[i].rearrange("(j p) d -> p j d", p=P))
            nc.sync.dma_start(vq_sb[P - 1:P, nt - 1, :], vnew[b:b + 1, h, :])

            # K requant (DVE) : m = absmax, c = 1/m, q = round(k*127*c)
            mk = sm.tile([P, nt], F32, tag="mk")
            nc.vector.tensor_reduce(mk, kq_sb, axis=mybir.AxisListType.X,
                                    op=AL.max, apply_absolute_value=True)
            ck = sm.tile([P, nt], F32, tag="ck")
            nc.vector.reciprocal(ck, mk)
            kqo = qop.tile([P, nt, HS], I8, tag="kqo")
            nc.vector.scalar_tensor_tensor(
                kqo, kq_sb, 127.0, ck[:, :, None].to_broadcast([P, nt, HS]),
                op0=AL.mult, op1=AL.mult)
            kso = sm.tile([P, nt], F32, tag="kso")
            nc.vector.scalar_tensor_tensor(kso, mk, 1.0 / 127.0, ks_ap,
                                           op0=AL.mult, op1=AL.mult)
            nc.sync.dma_start(kqo_d[i].rearrange("(j p) d -> p j d", p=P), kqo)
            nc.sync.dma_start(kso_d[i].rearrange("(j p) -> p j", p=P), kso)

            # V requant (absmax on DVE, big multiply on GPSIMD)
            mv = sm.tile([P, nt], F32, tag="mv")
            nc.vector.tensor_reduce(mv, vq_sb, axis=mybir.AxisListType.X,
                                    op=AL.max, apply_absolute_value=True)
            cv = sm.tile([P, nt], F32, tag="cv")
            nc.vector.reciprocal(cv, mv)
            cv127 = sm.tile([P, nt], F32, tag="cv127")
            nc.vector.tensor_scalar_mul(cv127, cv, 127.0)
            vqs = vp.tile([P, nt, HS], F32, tag="vqs")
            nc.gpsimd.tensor_tensor(
                vqs, vq_sb, cv127[:, :, None].to_broadcast([P, nt, HS]),
                op=AL.mult)
            vqo = qop.tile([P, nt, HS], I8, tag="vqo")
            nc.scalar.copy(vqo, vqs)
            vso = sm.tile([P, nt], F32, tag="vso")
            nc.vector.scalar_tensor_tensor(vso, mv, 1.0 / 127.0, vs_ap,
                                           op0=AL.mult, op1=AL.mult)
            nc.sync.dma_start(vqo_d[i].rearrange("(j p) d -> p j d", p=P), vqo)
            nc.sync.dma_start(vso_d[i].rearrange("(j p) -> p j", p=P), vso)

            # QK: per-tile PE transpose of k, then att[:, j] = kT_j.T @ q
            attp = attps.tile([P, nt], F32, tag="att")
            q_col = qkvT[:, h, b:b + 1]
            cs = 4 if nt % 4 == 0 else (2 if nt % 2 == 0 else 1)
            nchunk = nt // cs
            kts_prev = None
            for jc in range(nchunk + 1):
                if jc < nchunk:
                    ktp = ktps.tile([P, cs, HS], F32, tag="ktp")
                    for jj in range(cs):
                        nc.tensor.transpose(ktp[:, jj, :],
                                            kq_sb[:, jc * cs + jj, :], ident)
                    kts = ktsp.tile([P, cs, HS], F32, tag="kts")
                    nc.scalar.copy(kts, ktp)
                if jc > 0:
                    for jj in range(cs):
                        j = (jc - 1) * cs + jj
                        nc.tensor.matmul(attp[:, j:j + 1], lhsT=kts_prev[:, jj, :],
                                         rhs=q_col, start=True, stop=True)
                if jc < nchunk:
                    kts_prev = kts

            # logits = att * ks / sqrt(hs); u = exp(logits); zacc = row sums
            l = sm.tile([P, nt], F32, tag="l")
            nc.vector.scalar_tensor_tensor(l, attp, ISQ, ks_ap,
                                           op0=AL.mult, op1=AL.mult)
            u = sm.tile([P, nt], F32, tag="u")
            zacc = sm.tile([P, 1], F32, tag="zacc")
            nc.scalar.activation(u, l, mybir.ActivationFunctionType.Exp,
                                 accum_out=zacc)
            u2 = sm.tile([P, nt], F32, tag="u2")
            nc.vector.tensor_tensor(u2, u, vs_ap, op=AL.mult)
            s.update(vq_sb=vq_sb, u2=u2, zacc=zacc, h=h, b=b)
            return s

        def mid(s):
            # y_num = sum_t u2[t] * v[t, :]  (accumulated over tiles on PE)
            yz = yzps.tile([P, 4], F32, tag="yz")
            for j in range(nt):
                nc.tensor.matmul(yz[:, 0:1], lhsT=s["vq_sb"][:, j, :],
                                 rhs=s["u2"][:, j:j + 1],
                                 start=(j == 0), stop=(j == nt - 1))
            # Z = sum over partitions of zacc (ones matmul)
            nc.tensor.matmul(yz[0:1, 1:2], lhsT=ones_col, rhs=s["zacc"],
                             start=True, stop=True)
            s["yz"] = yz

        def tail(s):
            yz = s["yz"]
            rs = sm.tile([1, 1], F32, tag="rs")
            nc.vector.reciprocal(rs, yz[0:1, 1:2])
            nc.tensor.matmul(yz[:, 2:3], lhsT=ones_row, rhs=rs,
                             start=True, stop=True)
            yn_sb = sm.tile([P, 1], F32, tag="yn")
            nc.scalar.copy(yn_sb, yz[:, 0:1])
            nc.vector.tensor_tensor(yh[:, s["h"], s["b"]:s["b"] + 1],
                                    yn_sb, yz[:, 2:3], op=AL.mult)

        for i in range(NPAIR + 2):
            if i < NPAIR:
                st[i] = front(i)
            if 1 <= i <= NPAIR:
                mid(st[i - 1])
            if i >= 2:
                tail(st[i - 2])
                del st[i - 2]

        pair_ctx.close()

        # ---- phase 2: output projection (partial y) ----
        with tc.tile_pool(name="p2", bufs=1) as p2, \
             tc.tile_pool(name="p2ps", bufs=2, space="PSUM") as p2ps:
            ysb = p2.tile([B, C], F32)
            NTILE = 512
            for n0 in range(0, C, NTILE):
                yp = p2ps.tile([B, NTILE], F32, tag="yp")
                for hh in range(HPC):
                    nc.tensor.matmul(yp, lhsT=yh[:, hh, :],
                                     rhs=wp_sb[:, hh, n0:n0 + NTILE],
                                     start=(hh == 0), stop=(hh == HPC - 1))
                nc.scalar.copy(ysb[:, n0:n0 + NTILE], yp)
            nc.sync.dma_start(y_part, ysb)

    nc.compile()
    return nc


def _prep_core_inputs(c, nt, x, W_attn, b_attn, W_proj,
                      past_key_q, past_key_s, past_value_q, past_value_s):
    """Slice + repack full inputs for core c (heads 2c, 2c+1)."""
    T = nt * P
    h0 = HPC * c
    hsl = slice(h0 * HS, (h0 + HPC) * HS)
    wqkv = np.ascontiguousarray(
        np.concatenate([W_attn[:, hsl], W_attn[:, C:][:, hsl],
                        W_attn[:, 2 * C:][:, hsl]], axis=1))
    bq = np.concatenate([b_attn[hsl], b_attn[C:][hsl], b_attn[2 * C:][hsl]])
    wproj = np.ascontiguousarray(W_proj[hsl, :])

    def pack_cache(a):  # [B, HPC, T-1, HS] -> padded pair-major [NPAIR, T, HS]
        out = np.zeros((HPC * B, T, HS), dtype=a.dtype)
        out[:, :T - 1, :] = a.transpose(1, 0, 2, 3).reshape(HPC * B, T - 1, HS)
        return out

    kqc = pack_cache(past_key_q[:, h0:h0 + HPC, :T - 1, :])
    vqc = pack_cache(past_value_q[:, h0:h0 + HPC, :T - 1, :])
    scp = np.ones((HPC * B, 2, T), dtype=np.float32)
    scp[:, 0, :T - 1] = past_key_s[:, h0:h0 + HPC, :T - 1, 0].transpose(
        1, 0, 2).reshape(HPC * B, T - 1)
    scp[:, 1, :T - 1] = past_value_s[:, h0:h0 + HPC, :T - 1, 0].transpose(
        1, 0, 2).reshape(HPC * B, T - 1)
    return {"x": np.ascontiguousarray(x.reshape(B, C)),
            "wqkv": wqkv, "bqkv": bq, "wproj": wproj,
            "kq": kqc, "vq": vqc, "sc": scp}


def _assemble(results, nt, b_proj):
    T = nt * P
    y = np.zeros((B, C), dtype=np.float32)
    kq = np.empty((B, NH, T, HS), dtype=np.int8)
    ks = np.empty((B, NH, T, 1), dtype=np.float32)
    vq = np.empty((B, NH, T, HS), dtype=np.int8)
    vs = np.empty((B, NH, T, 1), dtype=np.float32)
    for c, r in enumerate(results):
        y += r["y_part"]
        h0 = HPC * c
        kq[:, h0:h0 + HPC] = r["kqo"].reshape(HPC, B, T, HS).transpose(1, 0, 2, 3)
        vq[:, h0:h0 + HPC] = r["vqo"].reshape(HPC, B, T, HS).transpose(1, 0, 2, 3)
        ks[:, h0:h0 + HPC] = r["kso"].reshape(HPC, B, T, 1).transpose(1, 0, 2, 3)
        vs[:, h0:h0 + HPC] = r["vso"].reshape(HPC, B, T, 1).transpose(1, 0, 2, 3)
    y = (y + b_proj).reshape(B, 1, C).astype(np.float32)
    return y, kq, ks, vq, vs


def make_in_maps(nt, **inputs):
    inputs = {k: np.asarray(v) for k, v in inputs.items()}
    return [_prep_core_inputs(c, nt, **inputs) for c in range(NCORES)]


def kernel(x, W_attn, b_attn, W_proj, b_proj,
           past_key_q, past_key_s, past_value_q, past_value_s):
    nt = 32
    in_maps = make_in_maps(
        nt, x=np.asarray(x, dtype=np.float32),
        W_attn=np.asarray(W_attn, dtype=np.float32),
        b_attn=np.asarray(b_attn, dtype=np.float32),
        W_proj=np.asarray(W_proj, dtype=np.float32),
        past_key_q=np.asarray(past_key_q, dtype=np.int32),
        past_key_s=np.asarray(past_key_s, dtype=np.float32),
        past_value_q=np.asarray(past_value_q, dtype=np.int32),
        past_value_s=np.asarray(past_value_s, dtype=np.float32))
    if nt not in _BUILD_CACHE:
        _BUILD_CACHE[nt] = build(nt)
    res = run_bass_kernel_spmd(_BUILD_CACHE[nt], in_maps,
                               core_ids=list(range(NCORES)))
    return _assemble([res.results[c] for c in range(NCORES)], nt,
                     np.asarray(b_proj, dtype=np.float32))
